# revision 73
# baseline (speedup 1.0000x reference)
"""AttnUpBlock2D Trainium2 kernel (collective-free, deinterleaved-W).

Pipeline per sample: bilinear up2 (align_corners) -> conv3x3(256->128)+BN+lrelu
-> conv3x3+BN+lrelu -> conv3x3+BN+lrelu -> self-attention (C=128, N=4096)
-> +identity -> lrelu.

Sharding: 8 cores = 4 samples x 2 query-halves. Each core computes the FULL
conv stack for its sample (convs are cheap relative to collective overhead),
then attention for 2048 query positions against the full local K/V. No
collectives. Query-half selection under one SPMD program: odd cores get the
sample VERTICALLY FLIPPED (x rows reversed, conv taps dy-reversed — exact for
align-corners bilinear and zero-padded conv); every core takes queries from
rows 0..31 of its local map and the host un-flips the output rows.

W axis is DEINTERLEAVED into even/odd halves (up/y0/y1 rows laid out as
[E: e0..e31,epad | Opad,o0..o31], 66 cols): the bilinear column interp then
writes contiguous runs (bf16 DVE 2x mode), and each 3x3 conv splits exactly
into per-half taps of the half-lattices (same total PE cost). Attention is
position-permutation-invariant; the host re-interleaves output columns.

Softmax uses a constant exp-shift (exact for any constant). exp output and V
are bf16 (err ~0.4%); attention row sums accumulate on the DVE in bf16
(partials of 32 values; final cross-partition reduce via a ones matmul) —
this keeps the PE free of the row-sum matmul stream. Logit path (convs, q/k)
stays float32r.

Schedule: the three conv layers run as a skewed wavefront (cu(G) | r0(G-1) |
r1(G-2)); as each r1 row group lands, its k/v/q projections and the attention
chunk work that just became runnable are emitted, so the ACT-bound exp stream
overlaps the PE-bound conv stream. PSUM (8 banks): 2 pS slots + 3 conv slots
+ 3 live pO accumulators (query tiles ms0..2). ms3 has no bank: its S^T/exp
rows are staged to SBUF (pt3) during the wavefront and its O matmuls run as a
burst from a conv slot once the convs finish (overlapping the final exps).

The K projection is fused away algebraically: softmax is over the key axis,
so the key bias cancels, and s[n,m] = y2_n^T (Wk^T Wq y2_m + Wk^T bq). The
host precomputes Wqk = Wk^T Wq; on device one fused projection qk = Wqk y2 +
Wk^T bq feeds S^T, whose stationary is y2 itself (full K=128).
"""

import os
import numpy as np
from ml_dtypes import bfloat16 as np_bf16

import concourse.bass as bass
import concourse.bacc as bacc
import concourse.tile as tile
from concourse import mybir
from concourse.bass_utils import run_bass_kernel_spmd

f32 = mybir.dt.float32
f32r = mybir.dt.float32r
bf16 = mybir.dt.bfloat16

B, CIN, C, HIN, WIN = 4, 256, 128, 32, 32
H, W = 64, 64                  # upsampled
N = H * W                      # 4096 positions per sample
HH = 32                        # query rows per core (half)
M = HH * W                     # 2048 own query positions per core
D = C // 2                     # 64 qk dim
EPS = 1e-5
ALPHA = 0.2
SHIFT = 40.0                   # exp shift; observed logit max ~53.6

UPR = 68                       # up rows t=0..67; image row j = t-3 (0..63 real)
XWR = 35                       # x window rows: idx r <-> x row r-2 (zeros OOB)
Y0R = 68                       # y0 rows: idx = image row + 2 (-2..65)
Y1R = 66                       # y1 rows: idx = image row + 1 (-1..64)
WB = 66                        # deinterleaved row: E 0..32 (pad@32), O 33..65 (pad@33)
HWIN = W // 2                  # 32 cols per half
N_CORES = 8

_PROGRAM = None
LAST_RUN = None


def build_program(reps=1):
    """reps>1 repeats the whole compute body (same output) — used only for
    differential hardware timing; the graded path uses reps=1."""
    global _PROGRAM
    if _PROGRAM is not None and reps == 1:
        return _PROGRAM

    nc = bacc.Bacc("TRN2", target_bir_lowering=False, debug=False,
                   num_devices=N_CORES)

    def din(name, shape, dt=f32):
        return nc.dram_tensor(name, list(shape), dt, kind="ExternalInput").ap()

    xw_ap = din("xw", [128, 2, XWR, WIN], bf16)
    wa_ap = din("wa", [128, UPR, WIN], bf16)     # row-interp weight, col-bcast
    wo_ap = din("wo", [128, WIN - 1], bf16)      # col-interp odd-half weights
    we_ap = din("we", [128, WIN - 1], bf16)      # col-interp even-half weights
    wu_ap = din("wu", [128, 2, 9, C], bf16)
    w0_ap = din("w0", [128, 9, C])
    w1_ap = din("w1", [128, 9, C])
    b0_ap = din("b0", [C, 1])
    b1_ap = din("b1", [C, 1])
    b2_ap = din("b2", [C, 1])
    wqt2_ap = din("wqt2", [C, 2, D])
    bq2_ap = din("bq2", [128, 1])
    wkt_ap = din("wkt", [C, D])
    bk2_ap = din("bk2", [128, 1])
    wvt2_ap = din("wvt2", [C, 2, C])
    bv_ap = din("bv", [C, 1])
    onesc_ap = din("onesc", [128, 1], bf16)
    out_ap = nc.dram_tensor("out", [C, HH, W], f32, kind="ExternalOutput").ap()

    with tile.TileContext(nc) as tc:
        from contextlib import ExitStack
        with ExitStack() as ctx:
          wp = ctx.enter_context(tc.tile_pool(name="wp", bufs=1))
          sb = ctx.enter_context(tc.tile_pool(name="sb", bufs=1))
          pts = ctx.enter_context(tc.tile_pool(name="pts", bufs=3))
          accp = ctx.enter_context(tc.tile_pool(name="accp", bufs=1))
          psS = ctx.enter_context(tc.tile_pool(name="psS", bufs=2, space="PSUM"))
          psO = ctx.enter_context(tc.tile_pool(name="psO", bufs=1, space="PSUM"))
          psC = ctx.enter_context(tc.tile_pool(name="psC", bufs=3, space="PSUM"))
          for _rep in range(reps):
              # ---- upsample scratch (freed after conv_up) ----
              upp_cm = tc.tile_pool(name=f"upp{_rep}", bufs=1)
              upp = upp_cm.__enter__()
              up = upp.tile([128, 2, UPR, WB], bf16)
              upx_cm = tc.tile_pool(name=f"upx{_rep}", bufs=1)
              upx = upx_cm.__enter__()
              xw = upx.tile([128, 2, XWR, WIN], bf16)
              nc.sync.dma_start(out=xw[:, :, 0:12, :], in_=xw_ap[:, :, 0:12, :])
              nc.sync.dma_start(out=xw[:, :, 12:XWR, :],
                                in_=xw_ap[:, :, 12:XWR, :])
              wa = wp.tile([128, UPR, WIN], bf16)
              nc.sync.dma_start(out=wa, in_=wa_ap)

              # ---- constants / weights ----
              wo = wp.tile([128, WIN - 1], bf16)
              nc.sync.dma_start(out=wo, in_=wo_ap)
              we = wp.tile([128, WIN - 1], bf16)
              nc.sync.dma_start(out=we, in_=we_ap)
              wu = wp.tile([128, 2, 9, C], bf16)
              nc.sync.dma_start(out=wu, in_=wu_ap)
              w0 = wp.tile([128, 9, C], f32r)
              nc.sync.dma_start(out=w0, in_=w0_ap.bitcast(f32r))
              w1t = wp.tile([128, 9, C], f32r)
              nc.sync.dma_start(out=w1t, in_=w1_ap.bitcast(f32r))
              b0 = wp.tile([C, 1], f32)
              nc.sync.dma_start(out=b0, in_=b0_ap)
              b1 = wp.tile([C, 1], f32)
              nc.sync.dma_start(out=b1, in_=b1_ap)
              b2 = wp.tile([C, 1], f32)
              nc.sync.dma_start(out=b2, in_=b2_ap)
              wqt2 = wp.tile([C, 2, D], f32r)
              nc.sync.dma_start(out=wqt2, in_=wqt2_ap.bitcast(f32r))
              wkt = wp.tile([C, D], f32r)
              nc.sync.dma_start(out=wkt, in_=wkt_ap.bitcast(f32r))
              wvt2 = wp.tile([C, 2, C], f32r)
              nc.sync.dma_start(out=wvt2, in_=wvt2_ap.bitcast(f32r))
              bq2 = wp.tile([128, 1], f32)
              nc.sync.dma_start(out=bq2, in_=bq2_ap)
              bk2 = wp.tile([128, 1], f32)
              nc.sync.dma_start(out=bk2, in_=bk2_ap)
              bv = wp.tile([C, 1], f32)
              nc.sync.dma_start(out=bv, in_=bv_ap)
              onesc = wp.tile([128, 1], bf16)
              nc.sync.dma_start(out=onesc, in_=onesc_ap)
              alpha = wp.tile([C, 1], f32)
              nc.vector.memset(alpha, ALPHA)
              alpha1 = wp.tile([C, 1], f32)
              nc.vector.memset(alpha1, 1.0)
              nshift = wp.tile([128, 1], f32)
              nc.vector.memset(nshift, -SHIFT)
              ones1 = wp.tile([1, 128], f32r)
              nc.vector.memset(ones1.bitcast(f32), 1.0)
              # preload the activation-function table while DMAs/upsample run
              actscr = wp.tile([1, 1], f32)
              nc.scalar.activation(out=actscr, in_=alpha[0:1, :],
                                   func=mybir.ActivationFunctionType.Exp,
                                   bias=nshift[0:1, :], scale=1.0)

              # ---- upsample rows (pairs (even,odd) t share xw rows t//2,
              # t//2+1; xw idx r <-> x row r-2, zeros OOB). wa encodes the
              # per-row lerp weight; rows t=0..2,67 come out exactly 0 via
              # wa (sources zero / w=1 cancels), no separate mask. ----
              dr = upx.tile([128, 2, XWR - 1, WIN], bf16)
              nc.vector.tensor_tensor(out=dr[:, :, 0:11, :],
                                      in0=xw[:, :, 1:12, :],
                                      in1=xw[:, :, 0:11, :],
                                      op=mybir.AluOpType.subtract)
              nc.vector.tensor_tensor(out=dr[:, :, 11:, :],
                                      in0=xw[:, :, 12:, :],
                                      in1=xw[:, :, 11:XWR - 1, :],
                                      op=mybir.AluOpType.subtract)
              xr = upx.tile([128, 2, UPR, WIN], bf16)
              # ---- blocked row+col interp (conv_up starts on early row
              # groups while later blocks interpolate) ----
              dc = upx.tile([128, 2, UPR, WIN - 1], bf16)
              nc.vector.memset(up[:, :, :, 32:34], 0.0)   # E pad, O pad cols
              for (rb0, rb1) in ((0, 18), (18, 34), (34, 52), (52, UPR)):
                  nrb = rb1 - rb0
                  na = nrb // 2
                  a0 = rb0 // 2
                  for ch in range(2):
                      def _pairs(t, row_stride, chunk_stride):
                          return bass.AP(
                              tensor=t.tensor,
                              offset=t.offset + ch * chunk_stride
                              + a0 * row_stride,
                              ap=[t.ap[0], [row_stride, na], [0, 2], [1, WIN]])
                      drv = _pairs(dr, WIN, (XWR - 1) * WIN)
                      xwv = _pairs(xw, WIN, XWR * WIN)
                      wav = bass.AP(tensor=wa.tensor,
                                    offset=wa.offset + rb0 * WIN,
                                    ap=[wa.ap[0], [2 * WIN, na], [WIN, 2],
                                        [1, WIN]])
                      xrc = xr[:, ch, rb0:rb1, :]
                      nc.vector.tensor_tensor(out=xrc, in0=drv, in1=wav,
                                              op=mybir.AluOpType.mult)
                      nc.vector.tensor_tensor(out=xrc, in0=xrc, in1=xwv,
                                              op=mybir.AluOpType.add)
                  nc.vector.tensor_tensor(out=dc[:, :, rb0:rb1, :],
                                          in0=xr[:, :, rb0:rb1, 1:],
                                          in1=xr[:, :, rb0:rb1, :-1],
                                          op=mybir.AluOpType.subtract)
                  for ch in range(2):
                      wo_b = bass.AP(tensor=wo.tensor, offset=wo.offset,
                                     ap=[wo.ap[0], [0, nrb], [1, WIN - 1]])
                      we_b = bass.AP(tensor=we.tensor, offset=we.offset,
                                     ap=[we.ap[0], [0, nrb], [1, WIN - 1]])
                      dcc = dc[:, ch, rb0:rb1, :]
                      xrc31 = xr[:, ch, rb0:rb1, 0:WIN - 1]
                      # O half: o[a] = xr[a] + wo[a]*dc[a] -> cols 34..64
                      uoc = up[:, ch, rb0:rb1, 34:65]
                      nc.vector.tensor_tensor(out=uoc, in0=dcc, in1=wo_b,
                                              op=mybir.AluOpType.mult)
                      nc.vector.tensor_tensor(out=uoc, in0=uoc, in1=xrc31,
                                              op=mybir.AluOpType.add)
                      # E half: e[a] = xr[a-1] + we[a-1]*dc[a-1] -> cols 1..31
                      uec = up[:, ch, rb0:rb1, 1:32]
                      nc.vector.tensor_tensor(out=uec, in0=dcc, in1=we_b,
                                              op=mybir.AluOpType.mult)
                      nc.vector.tensor_tensor(out=uec, in0=uec, in1=xrc31,
                                              op=mybir.AluOpType.add)
                      # exact cols: e[0]=xr[0] -> col 0; o[31]=xr[31] -> col 65
                      nc.vector.tensor_copy(up[:, ch, rb0:rb1, 0],
                                            xr[:, ch, rb0:rb1, 0])
                      nc.vector.tensor_copy(up[:, ch, rb0:rb1, 65],
                                            xr[:, ch, rb0:rb1, WIN - 1])

              # deinterleaved 3x3 tap windows: (src half, col offset) per
              # (out half, dx). layout col base: E=0, O=33 (pad at E32, O33).
              # E-out: dx-1 -> o[i-1] (cols 33..64), dx0 -> e[i] (0..31),
              #        dx+1 -> o[i] (34..65)
              # O-out: dx-1 -> e[i] (0..31), dx0 -> o[i] (34..65),
              #        dx+1 -> e[i+1] (1..32)
              TAPS = {0: (33, 0, 34), 1: (0, 34, 1)}

              upx_cm.__exit__(None, None, None)

              # ---- skewed conv wavefront + attention ----
              y0 = sb.tile([C, Y0R, WB], f32r)
              nc.vector.memset(y0[:, 0:2, :].bitcast(f32), 0.0)
              nc.vector.memset(y0[:, Y0R - 2:Y0R, :].bitcast(f32), 0.0)
              nc.vector.memset(y0[:, :, 32:34].bitcast(f32), 0.0)
              y1 = sb.tile([C, Y1R, WB], f32r)
              nc.vector.memset(y1[:, 0:1, :].bitcast(f32), 0.0)
              nc.vector.memset(y1[:, Y1R - 1:Y1R, :].bitcast(f32), 0.0)
              nc.vector.memset(y1[:, :, 32:34].bitcast(f32), 0.0)

              def conv_epilogue(oap, pt, bias):
                  # y = lrelu(psum + bias) on the DVE (keeps ACT free for exp)
                  bias_b = bass.AP(tensor=bias.tensor, offset=bias.offset,
                                   ap=[bias.ap[0], [0, 2], [0, 8], [0, HWIN]])
                  nc.vector.tensor_tensor(
                      out=oap, in0=pt.rearrange("p h (r w) -> p h r w", r=8),
                      in1=bias_b, op=mybir.AluOpType.add)
                  nc.vector.scalar_tensor_tensor(
                      out=oap, in0=oap, scalar=ALPHA, in1=oap,
                      op0=mybir.AluOpType.mult, op1=mybir.AluOpType.max)

              def conv_mv(src, ch_stride, ch, row0, dx):
                  # merged-half moving operand: both out halves of one tap
                  bE = TAPS[0][dx]
                  dlt = TAPS[1][dx] - TAPS[0][dx]
                  return bass.AP(tensor=src.tensor,
                                 offset=src.offset + ch * ch_stride
                                 + row0 * WB + bE,
                                 ap=[src.ap[0], [dlt, 2], [WB, 8], [1, HWIN]])

              def conv_up_group(g):
                  pt = psC.tile([C, 2, 256], f32, tag="cv", name="ptcu")
                  ptf = pt.rearrange("p h f -> p (h f)")
                  for dy in range(3):
                      for dx in range(3):
                          k = 3 * dy + dx
                          for ch in range(2):
                              nc.tensor.matmul(
                                  ptf, wu[:, ch, k, :],
                                  conv_mv(up, UPR * WB, ch, 8 * g + 2 + dy, dx),
                                  start=(k == 0 and ch == 0),
                                  stop=(k == 8 and ch == 1))
                  oap = bass.AP(tensor=y0.tensor,
                                offset=y0.offset + (8 * g + 2) * WB,
                                ap=[y0.ap[0], [34, 2], [WB, 8], [1, HWIN]])
                  nc.scalar.activation(out=oap,
                                       in_=pt.rearrange("p h (r w) -> p h r w", r=8),
                                       func=mybir.ActivationFunctionType.Prelu,
                                       bias=b0, scale=1.0, alpha=alpha)

              def r0_group(g):
                  pt = psC.tile([C, 2, 256], f32, tag="cv", name="ptr0")
                  ptf = pt.rearrange("p h f -> p (h f)")
                  for dy in range(3):
                      for dx in range(3):
                          k = 3 * dy + dx
                          nc.tensor.matmul(
                              ptf, w0[:, k, :],
                              conv_mv(y0, 0, 0, 8 * g + 1 + dy, dx),
                              start=(k == 0), stop=(k == 8))
                  oap = bass.AP(tensor=y1.tensor,
                                offset=y1.offset + (8 * g + 1) * WB,
                                ap=[y1.ap[0], [34, 2], [WB, 8], [1, HWIN]])
                  nc.scalar.activation(out=oap,
                                       in_=pt.rearrange("p h (r w) -> p h r w", r=8),
                                       func=mybir.ActivationFunctionType.Prelu,
                                       bias=b1, scale=1.0, alpha=alpha)

              # Attention unit (ms, cg) = S^T/exp/O/rowsum for query tile ms
              # over n-chunk group cg (chunks 4cg..4cg+3); needs q2 chunk ms
              # and k/v of group cg (both from y2 groups). Units are emitted
              # as soon as their r1 groups are done, cg ascending per ms, so
              # exp work (ACT-bound) overlaps conv matmuls (PE-bound).
              NJ = N // 128
              y2 = sb.tile([C, H, W], f32r)
              y2f = y2.rearrange("p r w -> p (r w)")
              q2 = sb.tile([128, M], f32r)       # q duplicated to 64..127
              k2 = sb.tile([128, N // 2], f32r)  # n-halves on partition halves
              vT = sb.tile([128, NJ, C], bf16)
              pt3 = sb.tile([128, NJ, 512], bf16)  # staged exp rows for ms=3
              pOs = {}
              accs = {}

              def attn_unit(ms, cg):
                  # chunk group cg (4 n-chunks) for query tile ms. For ms<3
                  # the O matmul accumulates into a live PSUM bank; ms==3
                  # only stages exp rows into pt3 (its O burst runs after
                  # tail(0) frees a bank). k2 keeps n-halves on partition
                  # halves; q2 is duplicated so both halves contract cleanly.
                  mlo = ms * 512
                  if ms not in accs:
                      if ms < 3:
                          pOs[ms] = psO.tile([C, 512], f32, tag=f"po{ms}",
                                             name=f"po{ms}")
                      accs[ms] = accp.tile([128, 512], bf16, tag=f"acc{ms}",
                                           name=f"acc{ms}")
                  acc = accs[ms]
                  for i in range(4):
                      j = 4 * cg + i
                      bp = 64 * (j // 16)
                      jj = j % 16
                      pS = psS.tile([128, 512], f32, tag="st")
                      nc.tensor.matmul(pS,
                                       k2[bp:bp + D, jj * 128:(jj + 1) * 128],
                                       q2[bp:bp + D, mlo:mlo + 512],
                                       start=True, stop=True)
                      if ms == 3:
                          pt = pt3[:, j, :]
                      else:
                          pt = pts.tile([128, 512], bf16, tag="pt")
                      nc.scalar.activation(out=pt, in_=pS,
                                           func=mybir.ActivationFunctionType.Exp,
                                           bias=nshift, scale=1.0)
                      if ms < 3:
                          nc.tensor.matmul(pOs[ms], vT[:, j, :], pt,
                                           start=(j == 0), stop=(j == NJ - 1))
                      if j == 0:
                          nc.vector.tensor_copy(acc, pt)
                      else:
                          nc.vector.tensor_tensor(out=acc, in0=acc, in1=pt,
                                                  op=mybir.AluOpType.add)

              bq_b = bass.AP(tensor=bq2.tensor, offset=bq2.offset,
                             ap=[bq2.ap[0], [0, 512]])
              def _bcast(col, bp=0, n=512):
                  s = col[bp:bp + D, :] if bp else col[0:D, :]
                  return bass.AP(tensor=s.tensor, offset=s.offset,
                                 ap=[s.ap[0], [0, n]])

              bk_b = {bp: _bcast(bk2, bp) for bp in (0, 64)}

              def r1_group(g):
                  # conv r1 group g -> y2 rows 8g..8g+8 (E 0..31, O 32..63)
                  pt = psC.tile([C, 2, 256], f32, tag="cv", name="ptr1")
                  ptf = pt.rearrange("p h f -> p (h f)")
                  for dy in range(3):
                      for dx in range(3):
                          k = 3 * dy + dx
                          nc.tensor.matmul(
                              ptf, w1t[:, k, :],
                              conv_mv(y1, 0, 0, 8 * g + dy, dx),
                              start=(k == 0), stop=(k == 8))
                  oap = bass.AP(tensor=y2.tensor, offset=y2.offset + 8 * g * W,
                                ap=[y2.ap[0], [HWIN, 2], [W, 8], [1, HWIN]])
                  nc.scalar.activation(out=oap,
                                       in_=pt.rearrange("p h (r w) -> p h r w", r=8),
                                       func=mybir.ActivationFunctionType.Prelu,
                                       bias=b2, scale=1.0, alpha=alpha)
                  # projections for this row group
                  c0 = g * 512
                  if g < 4:
                      pq = psS.tile([128, 512], f32, tag="st", name="pq")
                      nc.tensor.matmul(pq, wqt2.rearrange("p a b -> p (a b)"),
                                       y2f[:, c0:c0 + 512],
                                       start=True, stop=True)
                      nc.vector.tensor_tensor(out=q2[:, c0:c0 + 512], in0=pq,
                                              in1=bq_b, op=mybir.AluOpType.add)
                  bp, cc = (0, c0) if g < 4 else (64, c0 - N // 2)
                  pk = psS.tile([128, 512], f32, tag="st", name="pk")
                  nc.tensor.matmul(pk[0:D, :], wkt, y2f[:, c0:c0 + 512],
                                   start=True, stop=True)
                  if bp == 0:
                      nc.vector.tensor_tensor(out=k2[0:D, cc:cc + 512],
                                              in0=pk[0:D, :], in1=bk_b,
                                              op=mybir.AluOpType.add)
                  else:
                      # bias-add + partition shift in one ACT op
                      # (Prelu with alpha=1 is the identity)
                      nc.scalar.activation(
                          out=k2[64:128, cc:cc + 512], in_=pk[0:D, :],
                          func=mybir.ActivationFunctionType.Prelu,
                          bias=bk2[64:128, :], scale=1.0,
                          alpha=alpha1[0:D, :])
                  for j in range(4 * g, 4 * g + 4):
                      pv = psS.tile([128, 2, C], f32, tag="st", name="pv")
                      nc.tensor.matmul(pv, y2f[:, j * 128:(j + 1) * 128],
                                       wvt2.rearrange("p a b -> p (a b)"),
                                       start=True, stop=True)
                      nc.vector.tensor_copy(vT[:, j, :], pv[:, 0, :])

              def attn_tail(ms):
                  mlo = ms * 512
                  psums = psS.tile([1, 512], f32, tag="st", name="psums")
                  nc.tensor.matmul(psums, onesc, accs[ms], start=True, stop=True)
                  recip = pts.tile([1, 512], f32, tag="rc")
                  with nc.allow_low_precision(reason="softmax denominator"):
                      nc.vector.reciprocal(out=recip, in_=psums)
                  rbs = pts.tile([128, 512], f32, tag="rb")
                  nc.gpsimd.partition_broadcast(rbs, recip)
                  onorm = pts.tile([C, 512], f32, tag="on")
                  nc.vector.tensor_tensor(out=onorm, in0=pOs[ms], in1=rbs,
                                          op=mybir.AluOpType.mult)
                  # residual: + y2 rows + y0 rows (identity), same E/O order
                  y2v = bass.AP(tensor=y2.tensor, offset=y2.offset + 8 * ms * W,
                                ap=[y2.ap[0], [W, 8], [HWIN, 2], [1, HWIN]])
                  y0v = bass.AP(tensor=y0.tensor,
                                offset=y0.offset + (2 + 8 * ms) * WB,
                                ap=[y0.ap[0], [WB, 8], [34, 2], [1, HWIN]])
                  onv = onorm.rearrange("p (r h w) -> p r h w", r=8, h=2)
                  nc.vector.tensor_tensor(out=onv, in0=onv,
                                          in1=y2v.bitcast(f32),
                                          op=mybir.AluOpType.add)
                  nc.vector.tensor_tensor(out=onv, in0=onv,
                                          in1=y0v.bitcast(f32),
                                          op=mybir.AluOpType.add)
                  osb = pts.tile([C, 512], f32, tag="ob")
                  nc.scalar.activation(out=osb, in_=onorm,
                                       func=mybir.ActivationFunctionType.Prelu,
                                       bias=bv, scale=1.0, alpha=alpha)
                  nc.sync.dma_start(out=out_ap[:, ms * 8:(ms + 1) * 8, :],
                                    in_=osb.rearrange("p (r w) -> p r w", r=8))

              # skewed pipeline: conv_up(G) | r0(G-1) | r1(G-2). Attention
              # pass 1 (ms 0,1) rides the wavefront as k/v chunk groups
              # appear; pass 2 (ms 2,3) reuses the pass-1 PSUM banks after
              # their tails drain. This frees 2 PSUM banks so exp runs in
              # chunk pairs.
              for G in range(10):
                  if G < 8:
                      conv_up_group(G)
                  if 1 <= G <= 8:
                      r0_group(G - 1)
                  if G >= 2:
                      g = G - 2
                      r1_group(g)
                      units = []
                      if g <= 2:
                          units += [(g, cg) for cg in range(0, g)]
                          units += [(ms, g) for ms in range(0, min(g + 1, 3))]
                      else:
                          units += [(ms, g) for ms in range(3)]
                      if g == 3:
                          units += [(3, cg) for cg in range(4)]
                      elif g > 3:
                          units += [(3, g)]
                      for (ms, cg) in units:
                          attn_unit(ms, cg)

              upp_cm.__exit__(None, None, None)

              # deferred O burst for ms=3 (conv slots are free now; early
              # chunks run during the final exps)
              pOs[3] = psC.tile([C, 512], f32, tag="cv", name="po3")
              for j in range(NJ):
                  nc.tensor.matmul(pOs[3], vT[:, j, :], pt3[:, j, :],
                                   start=(j == 0), stop=(j == NJ - 1))
              for ms in range(4):
                  attn_tail(ms)

    nc.compile()
    if reps == 1:
        _PROGRAM = nc
    return nc


def _prep_inputs(x, W_up, b_up, g0, be0, m0, v0, W_r0, g1, be1, m1, v1,
                 W_r1, g2, be2, m2, v2, Wq, bq, Wk, bk, Wv, bv):
    """Build the 8 per-core input maps (host-side sharding/packing only)."""
    x = np.asarray(x, np.float32)

    def fold(wc, scale):
        return (wc * scale[:, None, None, None]).astype(np.float32)

    def pack(wc):  # [co, ci, 3, 3] -> [ci, 9, co]
        return np.ascontiguousarray(
            wc.transpose(1, 2, 3, 0).reshape(wc.shape[1], 9, wc.shape[0]))

    s0 = np.asarray(g0) / np.sqrt(np.asarray(v0) + EPS)
    s1 = np.asarray(g1) / np.sqrt(np.asarray(v1) + EPS)
    s2 = np.asarray(g2) / np.sqrt(np.asarray(v2) + EPS)
    b0f = (np.asarray(b_up) * s0 + np.asarray(be0) - np.asarray(m0) * s0)
    b1f = (np.asarray(be1) - np.asarray(m1) * s1)
    b2f = (np.asarray(be2) - np.asarray(m2) * s2)

    wu_f = fold(np.asarray(W_up), s0)            # [128, 256, 3, 3]
    w0_f = fold(np.asarray(W_r0), s1)
    w1_f = fold(np.asarray(W_r1), s2)

    def pack_u(wc):                              # -> [128, 2, 9, 128] bf16
        p = pack(wc)                             # [256, 9, 128]
        p = p.reshape(2, 128, 9, C).transpose(1, 0, 2, 3)
        return np.ascontiguousarray(p).astype(np_bf16)

    # h=1 cores get the sample vertically flipped => conv taps dy-reversed
    wu_p = {0: pack_u(wu_f), 1: pack_u(wu_f[:, :, ::-1, :])}
    w0_p = {0: np.ascontiguousarray(pack(w0_f), np.float32),
            1: np.ascontiguousarray(pack(w0_f[:, :, ::-1, :]), np.float32)}
    w1_p = {0: np.ascontiguousarray(pack(w1_f), np.float32),
            1: np.ascontiguousarray(pack(w1_f[:, :, ::-1, :]), np.float32)}

    co = np.linspace(0.0, HIN - 1.0, H)
    i0 = np.floor(co).astype(np.int64)
    wrow = (co - i0).astype(np.float32)

    # wa[t]: j = t-3; j=0 -> 1.0 (pair formula returns x[0] exactly);
    # t=0,1 sources are zero; t=2 (j=-1) -> 0; t=67 (j=64) -> 1.0 makes the
    # row exactly 0 (x[31] + 1.0*(0 - x[31])); broadcast along cols.
    w1a = np.zeros((UPR,), np.float32)
    for t in range(UPR):
        j = t - 3
        if j == 0:
            w1a[t] = 1.0
        elif 0 < j < H:
            w1a[t] = wrow[j]
        elif j == H:
            w1a[t] = 1.0
    wa_t = np.broadcast_to(w1a[:, None], (UPR, WIN))
    wa_t = np.broadcast_to(wa_t[None], (128, UPR, WIN)).astype(np_bf16)

    # col-interp weights: o[a] = lerp at v=2a+1 (w=wrow[2a+1]); e[a] at v=2a
    # (w=wrow[2a]); slots a=0..30 (plus unused last slot)
    wo_t = np.zeros((WIN - 1,), np.float32)
    we_t = np.zeros((WIN - 1,), np.float32)
    wo_t[0:31] = wrow[1:63:2]
    we_t[0:31] = wrow[2:64:2]
    wo_t = np.broadcast_to(wo_t[None], (128, WIN - 1)).astype(np_bf16)
    we_t = np.broadcast_to(we_t[None], (128, WIN - 1)).astype(np_bf16)

    wqt = np.asarray(Wq).T                       # [C, D]
    wqt2 = np.ascontiguousarray(
        np.stack([wqt, wqt], axis=1), np.float32)    # [C, 2, D]
    wkt = np.ascontiguousarray(np.asarray(Wk).T, np.float32)
    wvt = np.asarray(Wv).T
    wvt2 = np.ascontiguousarray(
        np.stack([wvt, wvt], axis=1), np.float32)    # [C, 2, C]
    bq2 = np.concatenate([np.asarray(bq)] * 2).astype(np.float32).reshape(128, 1)
    bk2 = np.concatenate([np.asarray(bk)] * 2).astype(np.float32).reshape(128, 1)
    bv_c = np.asarray(bv, np.float32).reshape(C, 1)
    b0c = b0f.astype(np.float32).reshape(C, 1)
    b1c = b1f.astype(np.float32).reshape(C, 1)
    b2c = b2f.astype(np.float32).reshape(C, 1)

    in_maps = []
    for core in range(N_CORES):
        s, h = core // 2, core % 2
        xs = x[s] if h == 0 else x[s][:, ::-1, :]    # [256, 32, 32]
        # xw idx r <-> x row r-2, zeros outside [0, 32)
        xw = np.zeros((CIN, XWR, WIN), np.float32)
        xw[:, 2:2 + HIN, :] = xs
        xw = np.ascontiguousarray(
            xw.reshape(2, 128, XWR, WIN).transpose(1, 0, 2, 3)).astype(np_bf16)
        in_maps.append(dict(
            xw=xw, wa=wa_t, wo=wo_t, we=we_t,
            wu=wu_p[h], w0=w0_p[h], w1=w1_p[h],
            b0=b0c, b1=b1c, b2=b2c,
            wqt2=wqt2, bq2=bq2, wkt=wkt, bk2=bk2, wvt2=wvt2, bv=bv_c,
            onesc=np.ones((128, 1), np.float32).astype(np_bf16),
        ))
    return in_maps


def kernel(**inputs):
    global LAST_RUN
    nc = build_program()
    in_maps = _prep_inputs(**inputs)
    trace = bool(int(os.environ.get("KERNEL_TRACE", "0")))
    res = run_bass_kernel_spmd(nc, in_maps, list(range(N_CORES)), trace=trace)
    LAST_RUN = res
    out = np.empty((B, C, H, W), np.float32)
    for core in range(N_CORES):
        s, h = core // 2, core % 2
        r = np.asarray(res.results[core]["out"])     # [C, 32, 64] E|O cols
        full = np.empty((C, HH, W), np.float32)
        full[:, :, 0::2] = r[:, :, 0:HWIN]
        full[:, :, 1::2] = r[:, :, HWIN:W]
        if h == 0:
            out[s, :, 0:HH, :] = full
        else:
            out[s, :, HH:H, :] = full[:, ::-1, :]
    return out
